# revision 17
# baseline (speedup 1.0000x reference)
"""GraphSAGE(max) 5-layer GNN on 8 Trainium2 NeuronCores.

Node-sharded graph parallelism; segment_max via int16 dma_gather over two
overlapping table windows (A=[0,32768), B=[17280,50048)); DVE max-folds with
a prefix-slot structure (destinations sorted by degree, slot k covers only
the first m_k nodes of a chunk); PE matmuls feature-major; AllGather rebuilds
the global node table between aggregation passes.

Only three gather passes run on the Q7 descriptor generators (the real
bottleneck at ~9.4ns/row):
  pass 1: agg(x)            -> mv    (rt1/mv1 share one aggregation of mv)
  pass 2: agg(mv)           -> rt, md (packed into one [50048,256] table)
  pass 3: agg([rt|md]) 512B -> r2, m2 (one gather serves both branches)
"""

import numpy as np

N_CORES = 8
N_NODES = 50000
F = 128
REAL_PER_CORE = N_NODES // N_CORES          # 6250
SPARE = 6
PER_CORE = REAL_PER_CORE + SPARE            # 6256
TOT_ROWS = N_CORES * PER_CORE               # 50048
WIN = 32768
WIN_B_BASE = TOT_ROWS - WIN                 # 17280
PAD_VAL = -60000.0
PAD_ROW = REAL_PER_CORE                     # abs row 6250 (core 0 spare 0)
ZERO_ROW = REAL_PER_CORE + 1                # abs row 6251 (core 0 spare 1)
PAD_ROW_B = 3 * PER_CORE + REAL_PER_CORE    # abs row 25018 (core 3 spare 0)
G_FULL = 512
CHUNKS = [G_FULL] * 12 + [PER_CORE - 12 * G_FULL]   # [512]*12 + [112]
MAX_CALL = 4096                             # max gathered rows per dma_gather


def _block_classes():
    """Per-position window eligibility, constant within each 512-chunk block
    (so within-chunk reordering never changes an edge's window class)."""
    p = np.arange(TOT_ROWS)
    local = p % PER_CORE
    blk = np.minimum(local // G_FULL, len(CHUNKS) - 1)
    base = (p // PER_CORE) * PER_CORE + blk * G_FULL
    width = np.where(blk == len(CHUNKS) - 1, PER_CORE - 12 * G_FULL, G_FULL)
    lo_b = base
    hi_b = base + width
    a_elig = hi_b <= WIN
    b_elig = lo_b >= WIN_B_BASE
    assert (a_elig | b_elig).all()
    return a_elig, b_elig


def _edge_counts(src_pos, dst_pos, a_elig, b_elig):
    """Per-destination window counts nA/nB with balanced overlap."""
    sa = a_elig[src_pos]
    sb = b_elig[src_pos]
    is_lo = sa & ~sb
    is_hi = sb & ~sa
    is_ov = sa & sb
    lo_cnt = np.bincount(dst_pos[is_lo], minlength=TOT_ROWS).astype(np.int32)
    ov_cnt = np.bincount(dst_pos[is_ov], minlength=TOT_ROWS).astype(np.int32)
    hi_cnt = np.bincount(dst_pos[is_hi], minlength=TOT_ROWS).astype(np.int32)
    empty = (lo_cnt + ov_cnt + hi_cnt) == 0
    lo2 = lo_cnt.copy()
    lo2[empty] = 1
    t = np.clip((hi_cnt + ov_cnt - lo2 + 1) // 2, 0, ov_cnt)
    nA = lo2 + t
    nB = hi_cnt + ov_cnt - t
    return nA, nB, t, empty, (is_lo, is_ov, is_hi)


def _snake_perm(deg, edge_index=None):
    order = np.argsort(-deg, kind="stable")
    r = np.arange(N_NODES)
    rnd, p8 = r // N_CORES, r % N_CORES
    core = np.where(rnd % 2 == 0, p8, N_CORES - 1 - p8)
    pos_of_rank = core * PER_CORE + rnd
    pos = np.empty(N_NODES, np.int64)
    pos[order] = pos_of_rank

    if edge_index is not None:
        # refine: sort real nodes within each (core, chunk) by max(nA, nB)
        # desc so the prefix-slot structure is tight.
        src, dst = np.asarray(edge_index[0]), np.asarray(edge_index[1])
        a_elig, b_elig = _block_classes()
        nA, nB, _, _, _ = _edge_counts(pos[src], pos[dst], a_elig, b_elig)
        key = np.maximum(nA, nB)  # keyed by position
        newpos = np.empty(TOT_ROWS, np.int64)
        chunk_start = [sum(CHUNKS[:i]) for i in range(len(CHUNKS))]
        for c in range(N_CORES):
            for ci, G in enumerate(CHUNKS):
                b0 = c * PER_CORE + chunk_start[ci]
                greal = G if ci < len(CHUNKS) - 1 else G - SPARE
                seg = np.arange(b0, b0 + greal)
                o = np.argsort(-key[seg], kind="stable")
                newpos[seg[o]] = seg
                if greal < G:
                    sp = np.arange(b0 + greal, b0 + G)
                    newpos[sp] = sp
        posmap = newpos  # old position -> new position
        pos = posmap[pos]

    node_by_pos = np.full(TOT_ROWS, -1, np.int64)
    node_by_pos[pos] = np.arange(N_NODES)
    return pos, node_by_pos


def _build_plan_and_indices(edge_index, pos):
    """Shared call plan (prefix-slot) + per-core int16 index arrays."""
    src, dst = np.asarray(edge_index[0]), np.asarray(edge_index[1])
    src_pos = pos[src]
    dst_pos = pos[dst]

    a_elig, b_elig = _block_classes()
    nA, nB, t, empty, _ = _edge_counts(src_pos, dst_pos, a_elig, b_elig)

    chunk_start = [sum(CHUNKS[:i]) for i in range(len(CHUNKS))]

    # prefix widths m_k per (chunk, window), max over cores
    def slot_widths(n_w):
        per_chunk = []
        for ci, G in enumerate(CHUNKS):
            off = chunk_start[ci]
            K = 0
            for c in range(N_CORES):
                base = c * PER_CORE + off
                K = max(K, int(n_w[base : base + G].max()))
            K = max(K, 1)
            ms = []
            for k in range(K):
                m = 0
                for c in range(N_CORES):
                    base = c * PER_CORE + off
                    nz = np.nonzero(n_w[base : base + G] >= k + 1)[0]
                    if nz.size:
                        m = max(m, int(nz[-1]) + 1)
                if m == 0:
                    m = G if k == 0 else 0
                if m > 0:
                    ms.append(m)
            per_chunk.append(ms)
        return per_chunk

    msA = slot_widths(nA)
    msB = slot_widths(nB)

    # batching into calls: per (chunk, window), greedy slots until MAX_CALL
    plan = []  # per chunk: calls (window, k0, [m...], col_off, cols, n_reg, n_static)
    col_off = 0
    for ci in range(len(CHUNKS)):
        calls = []
        for w, ms in (("A", msA[ci]), ("B", msB[ci])):
            k0 = 0
            while k0 < len(ms):
                tot = 0
                k1 = k0
                while k1 < len(ms) and tot + ms[k1] <= MAX_CALL:
                    tot += ms[k1]
                    k1 += 1
                n_reg = tot
                n_static = -(-n_reg // 128) * 128
                cols = -(-n_static // 16)
                calls.append((w, k0, ms[k0:k1], col_off, cols, n_reg, n_static))
                col_off += cols
                k0 = k1
        plan.append(calls)
    idx_cols = col_off

    # per-core index arrays
    idx_arrays = []
    for c in range(N_CORES):
        base = c * PER_CORE
        e_mask = (dst_pos >= base) & (dst_pos < base + PER_CORE)
        sp = src_pos[e_mask]
        dl = (dst_pos[e_mask] - base).astype(np.int64)
        o2 = np.argsort(dl, kind="stable")
        sp, dl = sp[o2], dl[o2]
        lo_m = a_elig[sp] & ~b_elig[sp]
        hi_m = b_elig[sp] & ~a_elig[sp]
        ov_m = a_elig[sp] & b_elig[sp]
        ov_d = dl[ov_m]
        ov_rank = np.arange(ov_d.size) - np.searchsorted(ov_d, ov_d, side="left")
        ov_toA = ov_rank < t[base + ov_d]
        a_vals = np.concatenate([sp[lo_m], sp[ov_m][ov_toA]]).astype(np.int32)
        a_dsts = np.concatenate([dl[lo_m], ov_d[ov_toA]]).astype(np.int64)
        b_vals = np.concatenate([sp[hi_m], sp[ov_m][~ov_toA]]).astype(np.int32) - WIN_B_BASE
        b_dsts = np.concatenate([dl[hi_m], ov_d[~ov_toA]]).astype(np.int64)

        def mk_window(wvals, wdsts):
            o3 = np.argsort(wdsts, kind="stable")
            wv, wd = wvals[o3], wdsts[o3]
            rank = np.arange(wd.size) - np.searchsorted(wd, wd, side="left")
            return wv, wd, rank

        av, ad, ar = mk_window(a_vals, a_dsts)
        bv, bd, br = mk_window(b_vals, b_dsts)

        idx_arr = np.zeros((128, idx_cols), np.int16)
        for ci, G in enumerate(CHUNKS):
            off = chunk_start[ci]
            mats = {}
            for wname, (wv, wd, wr), ms, pad_idx in (
                ("A", (av, ad, ar), msA[ci], PAD_ROW),
                ("B", (bv, bd, br), msB[ci], PAD_ROW_B - WIN_B_BASE),
            ):
                K = len(ms)
                M = np.full((K, G), pad_idx, np.int32)
                m = (wd >= off) & (wd < off + G)
                sel = wr[m] < K
                M[wr[m][sel], wd[m][sel] - off] = wv[m][sel]
                if wname == "A":
                    je = np.nonzero(empty[base + off : base + off + G])[0]
                    M[0, je] = ZERO_ROW
                mats[wname] = M
            for (w2, k0, ms_c, co, cols, n_reg, n_static) in plan[ci]:
                M = mats[w2]
                parts = [M[k0 + i, :m] for i, m in enumerate(ms_c)]
                flat = np.full(n_static, -1, np.int32)
                cat = np.concatenate(parts)
                flat[: cat.size] = cat
                assert cat.size == n_reg
                assert flat.max() < WIN and flat.min() >= -1
                blk = flat.astype(np.int16).reshape(cols, 16).T
                idx_arr[0:16, co : co + cols] = blk
                idx_arr[16:32, co : co + cols] = blk
        idx_arrays.append(idx_arr)

    tot_rows = sum(sum(c[5] for c in calls) for calls in plan)
    stats = {
        "edges": int(src.size),
        "rows_per_pass": tot_rows,
        "idx_cols": idx_cols,
        "n_calls": sum(len(c) for c in plan),
    }
    return plan, idx_arrays, idx_cols, stats


def _build_nc(plan, idx_cols, stages=5):
    import concourse.bass as bass
    import concourse.mybir as mybir
    import concourse.tile as tile
    from concourse import bacc
    from concourse.masks import make_identity

    fp16 = mybir.dt.float16
    f32 = mybir.dt.float32
    Relu = mybir.ActivationFunctionType.Relu
    Copy = mybir.ActivationFunctionType.Copy
    MAX = mybir.AluOpType.max

    nc = bacc.Bacc("TRN2", num_devices=N_CORES, dynamic_dma_scratch_size=40960)

    xtab = nc.dram_tensor("xtab", [TOT_ROWS, F], fp16, kind="ExternalInput")
    xloc = nc.dram_tensor("xloc", [F, PER_CORE], fp16, kind="ExternalInput")
    idx_in = nc.dram_tensor("idx", [128, idx_cols], mybir.dt.int16, kind="ExternalInput")
    wpack = nc.dram_tensor("wpack", [F, 10 * F + 2], fp16, kind="ExternalInput")
    bpack = nc.dram_tensor("bpack", [F, 7], f32, kind="ExternalInput")
    cpad = nc.dram_tensor("cpad", [SPARE, F], fp16, kind="ExternalInput")
    out2 = nc.dram_tensor("out2", [2, PER_CORE], f32, kind="ExternalOutput")
    dbg = nc.dram_tensor("dbg", [F, PER_CORE], fp16, kind="ExternalOutput")

    a_mv = nc.dram_tensor("agin_mv", [PER_CORE, F], fp16)
    a_comb = nc.dram_tensor("agin_comb", [PER_CORE, 2 * F], fp16)
    tab_mv = nc.dram_tensor("tab_mv", [TOT_ROWS, F], fp16, addr_space="Shared")
    tab_comb = nc.dram_tensor("tab_comb", [TOT_ROWS, 2 * F], fp16, addr_space="Shared")

    LCOL = {"S": 0, "rt1": 2, "rt2": 4, "mv1": 6, "mv2": 8}
    BCOL = {"S": 0, "rt1": 1, "rt2": 2, "mv1": 3, "mv2": 4}

    with tile.TileContext(nc) as tc:
        with (
            tc.tile_pool(name="persist", bufs=1) as pp,
            tc.tile_pool(name="gb", bufs=4) as gbp,
            tc.tile_pool(name="acc", bufs=3) as accp,
            tc.tile_pool(name="om", bufs=2) as omp,
            tc.tile_pool(name="psT", bufs=2, space="PSUM") as psTp,
            tc.tile_pool(name="psN", bufs=3, space="PSUM") as psNp,
            tc.tile_pool(name="psF", bufs=2, space="PSUM") as psFp,
        ):
            idx_t = pp.tile([128, idx_cols], mybir.dt.int16, tag="idx", name="idx")
            nc.sync.dma_start(out=idx_t[:], in_=idx_in[:, :])
            w_t = pp.tile([F, 10 * F + 2], fp16, tag="w", name="w")
            nc.sync.dma_start(out=w_t[:], in_=wpack[:, :])
            b_t = pp.tile([F, 7], f32, tag="b", name="b")
            nc.sync.dma_start(out=b_t[:], in_=bpack[:, :])
            ident = pp.tile([F, F], fp16, tag="ident", name="ident")
            make_identity(nc, ident[:])

            x_t = pp.tile([F, PER_CORE], fp16, tag="xloc", name="xloc_t")
            nc.sync.dma_start(out=x_t[:], in_=xloc[:, :])

            locs = {k: pp.tile([F, PER_CORE], fp16, tag=f"loc_{k}", name=f"loc_{k}")
                    for k in ("mv", "rt", "md", "r2", "m2")}

            def wsl(lname):
                return (w_t[:, LCOL[lname] * F : (LCOL[lname] + 1) * F],
                        w_t[:, (LCOL[lname] + 1) * F : (LCOL[lname] + 2) * F],
                        b_t[:, BCOL[lname] : BCOL[lname] + 1])

            def agg_chunk(ci, table, planes, accs):
                """Gather + max-fold one chunk; planes=1 or 2."""
                G = CHUNKS[ci]
                es = 128 * planes
                winA = table[0:WIN, :]
                winB = table[WIN_B_BASE : WIN_B_BASE + WIN, :]
                for a in accs:
                    nc.vector.memset(a[:, :G], PAD_VAL)
                for (w2, k0, ms_c, co, cols, n_reg, n_static) in plan[ci]:
                    gb = gbp.tile([F, planes * n_static], fp16, tag="gb", name="gb",
                                  padded_shape=[F, 2 * MAX_CALL])
                    nc.gpsimd.dma_gather(
                        gb[:, : planes * n_static].rearrange(
                            "p (q n) -> p q n", q=planes),
                        winA if w2 == "A" else winB,
                        idx_t[:, co : co + cols],
                        n_static, n_reg, es,
                        transpose=True, single_packet=False,
                    )
                    off_k = 0
                    for m in ms_c:
                        for p, a in enumerate(accs):
                            nc.vector.tensor_tensor(
                                out=a[:, :m], in0=a[:, :m],
                                in1=gb[:, p * n_static + off_k : p * n_static + off_k + m],
                                op=MAX)
                        off_k += m

            def mm_epilogue(ci, off, acc, xT, lname, out_tile):
                G = CHUNKS[ci]
                wl, wr, bias = wsl(lname)
                psT = psTp.tile([F, G_FULL], f32, tag="psT", name="psT")
                nc.tensor.matmul(out=psT[:, :G], lhsT=wl, rhs=acc[:, :G],
                                 start=True, stop=False)
                nc.tensor.matmul(out=psT[:, :G], lhsT=wr, rhs=xT[:, off : off + G],
                                 start=False, stop=True)
                nc.scalar.activation(out=out_tile[:, off : off + G], in_=psT[:, :G],
                                     func=Relu, bias=bias, scale=1.0)

            def to_node_major(ci, off, out_tile, dst_dram, dcol):
                G = CHUNKS[ci]
                ngroups = -(-G // 128)
                om = omp.tile([128, 4 * 128], fp16, tag="om", name="om")
                for g in range(ngroups):
                    gw = min(128, G - g * 128)
                    psN = psNp.tile([128, 128], fp16, tag="psN", name="psN")
                    nc.tensor.transpose(
                        out=psN[:gw, :],
                        in_=out_tile[:, off + g * 128 : off + g * 128 + gw],
                        identity=ident[:])
                    nc.scalar.activation(out=om[:gw, g * 128 : (g + 1) * 128],
                                         in_=psN[:gw, :], func=Copy,
                                         bias=0.0, scale=1.0)
                if ci == len(CHUNKS) - 1:
                    nc.sync.dma_start(out=om[106:112, 0:128], in_=cpad[:, :])
                if G == G_FULL:
                    nc.sync.dma_start(
                        out=dst_dram[off : off + G, dcol : dcol + 128].rearrange(
                            "(g p) f -> p g f", p=128),
                        in_=om[:].rearrange("p (g f) -> p g f", g=4))
                else:
                    nc.sync.dma_start(out=dst_dram[off : off + G, dcol : dcol + 128],
                                      in_=om[:G, 0:128])

            def allgather(a_in, tab):
                nc.gpsimd.collective_compute(
                    "AllGather", mybir.AluOpType.bypass,
                    replica_groups=[list(range(N_CORES))],
                    ins=[a_in[:, :]], outs=[tab[:, :]])

            # ---- pass 1: agg(x) -> mv ----
            off = 0
            for ci, G in enumerate(CHUNKS):
                acc = accp.tile([F, G_FULL], fp16, tag="acc", name="acc")
                agg_chunk(ci, xtab, 1, [acc])
                mm_epilogue(ci, off, acc, x_t, "S", locs["mv"])
                if stages >= 2:
                    to_node_major(ci, off, locs["mv"], a_mv, 0)
                off += G
            if stages >= 2:
                allgather(a_mv, tab_mv)

            # ---- pass 2: agg(mv) -> rt, md ----
            if stages >= 3:
                off = 0
                for ci, G in enumerate(CHUNKS):
                    acc = accp.tile([F, G_FULL], fp16, tag="acc", name="acc")
                    agg_chunk(ci, tab_mv, 1, [acc])
                    mm_epilogue(ci, off, acc, locs["mv"], "rt1", locs["rt"])
                    mm_epilogue(ci, off, acc, locs["mv"], "mv1", locs["md"])
                    to_node_major(ci, off, locs["rt"], a_comb, 0)
                    to_node_major(ci, off, locs["md"], a_comb, 128)
                    off += G
                allgather(a_comb, tab_comb)

            # ---- pass 3: agg([rt|md]) -> r2, m2 ----
            if stages >= 4:
                off = 0
                for ci, G in enumerate(CHUNKS):
                    accR = accp.tile([F, G_FULL], fp16, tag="accR", name="accR")
                    accM = accp.tile([F, G_FULL], fp16, tag="accM", name="accM")
                    agg_chunk(ci, tab_comb, 2, [accR, accM])
                    mm_epilogue(ci, off, accR, locs["rt"], "rt2", locs["r2"])
                    mm_epilogue(ci, off, accM, locs["md"], "mv2", locs["m2"])
                    off += G

            # ---- finals ----
            if stages >= 5:
                rtw = w_t[:, 10 * F : 10 * F + 1]
                mvw = w_t[:, 10 * F + 1 : 10 * F + 2]
                rtb = b_t[0:1, 5:6]
                mvb = b_t[0:1, 6:7]
                off = 0
                for ci, G in enumerate(CHUNKS):
                    for row, (wv, bv, srck) in enumerate(
                        ((rtw, rtb, "r2"), (mvw, mvb, "m2"))
                    ):
                        psF = psFp.tile([1, G_FULL], f32, tag="psF", name="psF")
                        nc.tensor.matmul(out=psF[:1, :G], lhsT=wv,
                                         rhs=locs[srck][:, off : off + G],
                                         start=True, stop=True)
                        fbuf = gbp.tile([1, G_FULL], f32, tag="fbuf", name="fbuf", bufs=2)
                        nc.vector.tensor_scalar(
                            out=fbuf[0:1, :G], in0=psF[:1, :G],
                            scalar1=bv, scalar2=None, op0=mybir.AluOpType.add)
                        nc.sync.dma_start(out=out2[row : row + 1, off : off + G],
                                          in_=fbuf[0:1, :G])
                    off += G

            dbg_src = {1: "mv", 2: "mv", 3: "rt", 4: "r2", 5: "r2"}[stages]
            nc.sync.dma_start(out=dbg[:, :], in_=locs[dbg_src][:])
            if stages < 4:
                for kk in ("r2", "m2"):
                    nc.vector.memset(locs[kk][:, :], 0.0)

    nc.finalize()
    return nc


_CACHE = {}
_TRACE = False
_LAST_RESULT = None


def kernel(x, edge_index, shared_Wl, shared_b, shared_Wr,
           rt1_Wl, rt1_b, rt1_Wr, rt2_Wl, rt2_b, rt2_Wr, rt3_W, rt3_b,
           mv1_Wl, mv1_b, mv1_Wr, mv2_Wl, mv2_b, mv2_Wr, mv3_W, mv3_b):
    from concourse.bass_utils import run_bass_kernel_spmd

    x = np.asarray(x)
    edge_index = np.asarray(edge_index)

    key = hash(edge_index.tobytes())
    if key not in _CACHE:
        deg = np.bincount(edge_index[1], minlength=N_NODES)
        pos, node_by_pos = _snake_perm(deg, edge_index)
        plan, idx_arrays, idx_cols, stats = _build_plan_and_indices(edge_index, pos)
        nc = _build_nc(plan, idx_cols)
        _CACHE[key] = (pos, node_by_pos, plan, idx_arrays, idx_cols, stats, nc)
    pos, node_by_pos, plan, idx_arrays, idx_cols, stats, nc = _CACHE[key]

    xtab = np.empty((TOT_ROWS, F), np.float16)
    real = node_by_pos >= 0
    xtab[real] = x[node_by_pos[real]].astype(np.float16)
    for c in range(N_CORES):
        b0 = c * PER_CORE + REAL_PER_CORE
        xtab[b0 : b0 + SPARE] = PAD_VAL
        xtab[b0 + 1] = 0.0

    def t16(w):
        return np.ascontiguousarray(np.asarray(w).T.astype(np.float16))

    wpack = np.concatenate(
        [t16(shared_Wl), t16(shared_Wr), t16(rt1_Wl), t16(rt1_Wr),
         t16(rt2_Wl), t16(rt2_Wr), t16(mv1_Wl), t16(mv1_Wr),
         t16(mv2_Wl), t16(mv2_Wr), t16(rt3_W), t16(mv3_W)], axis=1)
    bpack = np.zeros((F, 7), np.float32)
    for i, b in enumerate((shared_b, rt1_b, rt2_b, mv1_b, mv2_b)):
        bpack[:, i] = np.asarray(b, np.float32)
    bpack[0, 5] = float(np.asarray(rt3_b).reshape(-1)[0])
    bpack[0, 6] = float(np.asarray(mv3_b).reshape(-1)[0])

    cpad_arr = np.full((SPARE, F), PAD_VAL, np.float16)
    cpad_arr[1] = 0.0

    in_maps = []
    for c in range(N_CORES):
        sl = slice(c * PER_CORE, (c + 1) * PER_CORE)
        xloc = np.ascontiguousarray(xtab[sl].T)
        xloc[:, REAL_PER_CORE:] = 0
        in_maps.append({
            "xtab": xtab, "xloc": xloc, "idx": idx_arrays[c],
            "wpack": wpack, "bpack": bpack, "cpad": cpad_arr,
        })

    global _LAST_RESULT
    res = run_bass_kernel_spmd(nc, in_maps, core_ids=list(range(N_CORES)),
                               trace=_TRACE)
    _LAST_RESULT = res

    rtAngle = np.empty(N_NODES, np.float32)
    moveDis = np.empty(N_NODES, np.float32)
    for c in range(N_CORES):
        o = res.results[c]["out2"]
        nodes = node_by_pos[c * PER_CORE : c * PER_CORE + REAL_PER_CORE]
        rtAngle[nodes] = o[0, :REAL_PER_CORE]
        moveDis[nodes] = o[1, :REAL_PER_CORE]
    return (rtAngle, moveDis)
